# revision 26
# baseline (speedup 1.0000x reference)
"""Causal single-head attention on 8 TRN2 NeuronCores, v5 (DoubleRow + fp16 scores).

Problem: K,Q,V [4, 4096, 1024] f32, Wk/Wq/Wv [1024, 64] f32.
out[b,q,:] = softmax_causal((Q Wq)(K Wk)^T / 8) @ (V Wv)

Sharding: core c = 2b+h owns batch b = c//2 with key-parity h = c%2 (local
key block l = global block 2l+h). Each core emits numerator+denominator
[4096, 65]; the host sums the pair and divides.

Engine strategy: the exp stream on ScalarE (~36us busy) is the hard floor;
everything else is compressed far below it and scheduled around keeping
ScalarE fed:
 - x ships entirely as fp8e4 (IEEE e4m3, max 240) host-folded [128, 2, ...]
   so projections run DoubleRow with 256-row contraction (4x fewer PE
   cycles); weights are 32x-scaled so all on-chip values stay within e4m3
   range.
 - q/k are stored fp16 (scores wear quantization noise directly, so fp8
   storage would blow the 2e-2 error budget); QK is a plain fp16 matmul.
 - pe (exp output) is written e4m3 and PV pairs two key blocks per
   DoubleRow matmul (pe group slots are j-adjacent; v8 is padded to 80
   cols so slot strides stay %16==0); odd tails pair with zero slots.
 - the e5m2 DoubleRow mask matmul folds the causal diagonal into the score
   accumulation; exp is exp(score * 2^-13 - 2.5) on ScalarE.
All DRAM tensors are host-packed chunk-contiguous (one descriptor per
partition, full DMA rate); chunk sizes/order are tuned so each lands just
before its consumer block (DMA_ENGINES is a serial resource in the cost
model). The final block's output ships straight from PSUM to cut the tail.
"""

import ml_dtypes
import numpy as np

import concourse.mybir as mybir
import concourse.tile as tile
from concourse import bacc
from concourse.bass_utils import run_bass_kernel_spmd

B, T, E, D = 4, 4096, 1024, 64
NCORES = 8
QB = 256        # query block
KB = 128        # key block
NQB = T // QB   # 16 query blocks
NLK = T // KB // 2   # 16 local key blocks per core
G = 4           # key blocks per PSUM score group / exp call
EP = E // 256   # 4 folded e-pairs
I16 = 2         # blocks 0-1 use fp16 pe/v in PV (early-query insurance)
L16 = 2

F32 = mybir.dt.float32
F16 = mybir.dt.float16
E4M3 = mybir.dt.float8e4
E5M2 = mybir.dt.float8e5
DRow = mybir.MatmulPerfMode.DoubleRow

WSCALE = 32.0
EXP_SCALE = 0.125 / (WSCALE * WSCALE)   # 2^-13, exact
EXP_BIAS = -2.5
MASK_VAL = -14336.0   # e5m2-exact; * 1024 (ident) * 2^-13 = -1792 -> exp = 0

_CACHE = {}

PV8 = True    # e4m3 exp output + DoubleRow paired PV for blocks >= I16

# chunk column layouts (local cols). Early cols ship fp16 (early rows have
# large |out| and little softmax averaging, so they set the max-err); the
# bulk ships folded e4m3. Early chunks small so phase 0 starts fast.
K_HI = [128, 128]                     # kb0, kb1 (fp16)
Q_HI = [256, 256]                     # blocks 0, 1 (fp16)
V_HI = [256]                          # kb0-1 (fp16)
K_CH = [256, 512, 512, 512]           # kb2-3, 4-7, 8-11, 12-15 (e4m3)
Q_CH = [256] * 14                     # blocks 2..15, one chunk each
V_CH = [256, 256, 512, 512, 512]      # kb0-1(v8p), 2-3, 4-7, 8-11, 12-15
KHI = sum(K_HI)
QHI = sum(Q_HI)
VHI = sum(V_HI)


def _build_nc():
    nc = bacc.Bacc()
    kh_d = nc.declare_dram_parameter("kh", [128, 8 * KHI], F16, isOutput=False)
    vh_d = nc.declare_dram_parameter("vh", [128, 8 * VHI], F16, isOutput=False)
    qh_d = nc.declare_dram_parameter("qh", [128, 8 * QHI], F16, isOutput=False)
    qt_d = nc.declare_dram_parameter("qt", [128, 2 * EP * (T - QHI)], E4M3, isOutput=False)
    kt_d = nc.declare_dram_parameter("kt", [128, 2 * EP * (T // 2 - KHI)], E4M3, isOutput=False)
    vt_d = nc.declare_dram_parameter("vt", [128, 2 * EP * (T // 2)], E4M3, isOutput=False)
    wb_d = nc.declare_dram_parameter("wb", [128, 3 * 8 * D], F16, isOutput=False)
    w8_d = nc.declare_dram_parameter("w8", [128, 2 * 3 * EP * D], E4M3, isOutput=False)
    idm_d = nc.declare_dram_parameter("idm", [64, 2 * 128], E5M2, isOutput=False)
    mask_d = nc.declare_dram_parameter("mask", [64, 2 * QB], E5M2, isOutput=False)
    on_d = nc.declare_dram_parameter("on", [128, NQB * 2 * D], mybir.dt.bfloat16, isOutput=True)
    od_d = nc.declare_dram_parameter("od", [128, NQB * 2], F32, isOutput=True)

    with tile.TileContext(nc) as tc:
        with (
            tc.tile_pool(name="w", bufs=1) as wpool,
            tc.tile_pool(name="res", bufs=1) as res,
            tc.tile_pool(name="stage", bufs=1) as stage,
            tc.tile_pool(name="pe16", bufs=2) as pe16_pool,
            tc.tile_pool(name="pe8", bufs=7) as pe8_pool,
            tc.tile_pool(name="psP", bufs=2, space="PSUM") as psP,
            tc.tile_pool(name="psA", bufs=2, space="PSUM") as psA,
            tc.tile_pool(name="psO", bufs=2, space="PSUM") as psO,
        ):
            w8 = wpool.tile([128, 2, 3, EP, D], E4M3, tag="w8")
            wb = wpool.tile([128, 3, 8, D], F16, tag="wb")
            idm = wpool.tile([64, 2, 128], E5M2, tag="idm")
            mask_sb = wpool.tile([64, 2, QB], E5M2, tag="mask")
            bias_sb = wpool.tile([128, 1], F32, tag="bias")
            warm_sb = wpool.tile([64, 16], F16, tag="warm")
            nc.vector.memset(bias_sb[:], EXP_BIAS)
            nc.vector.memset(warm_sb[:], 0.0)

            qT16 = res.tile([64, T], F16, tag="qT16")
            kT16 = res.tile([64, T // 2], F16, tag="kT16")
            v16 = res.tile([128, L16, D + 1], F16, tag="v16")
            v8p = res.tile([128, NLK + 1, 80], E4M3, tag="v8p")
            on_sb = res.tile([128, NQB * 2, D], mybir.dt.bfloat16, tag="on")
            od_sb = res.tile([128, NQB * 2], F32, tag="od")

            # PE warm-up: tiny matmuls at t~0 so the p-state ramp (3us from
            # first PE activity) finishes before the first real matmul.
            ps_warm = psP.tile([128, 512], F32, tag="ps")
            for r in range(6):
                nc.tensor.matmul(
                    ps_warm[:16, :16], lhsT=warm_sb[:, :16], rhs=warm_sb[:, :16],
                    start=True, stop=True,
                )

            nc.vector.memset(v16[:, :, D : D + 1], 1.0)
            nc.vector.memset(v8p[:, 0:NLK, D : D + 1], 1.0)
            nc.vector.memset(v8p[:, NLK, :], 0.0)

            nc.gpsimd.dma_start(out=idm[:], in_=idm_d.rearrange("p (two m) -> p two m", two=2))
            nc.gpsimd.dma_start(out=mask_sb[:], in_=mask_d.rearrange("p (two m) -> p two m", two=2))

            def load16(src_d, name, c0, cols):
                raw = stage.tile([128, 8, cols], F16, tag=f"{name}h{c0}")
                off = 8 * c0
                nc.sync.dma_start(
                    out=raw[:],
                    in_=src_d[:, off : off + 8 * cols].rearrange(
                        "p (i t) -> p i t", i=8),
                )
                return raw

            def proj_qk_hi(raw, wi, dst16, col0, cols):
                ps = psP.tile([128, 512], F32, tag="ps")
                for i in range(8):
                    nc.tensor.matmul(
                        ps[:D, :cols],
                        lhsT=wb[:, wi, i, :],
                        rhs=raw[:, i, :],
                        start=(i == 0),
                        stop=(i == 7),
                    )
                nc.vector.tensor_copy(dst16[:, col0 : col0 + cols], ps[:D, :cols])

            def proj_v_hi(raw, lk0, nkb):
                for t in range(nkb):
                    ps = psP.tile([128, 512], F32, tag="ps")
                    for i in range(8):
                        nc.tensor.matmul(
                            ps[:, :D],
                            lhsT=raw[:, i, t * KB : (t + 1) * KB],
                            rhs=wb[:, 1, i, :],
                            start=(i == 0),
                            stop=(i == 7),
                        )
                    if lk0 + t < L16:
                        nc.vector.tensor_copy(v16[:, lk0 + t, :D], ps[:, :D])
                    nc.vector.tensor_copy(v8p[:, lk0 + t, :D], ps[:, :D])

            def load8(src_d, name, c0, cols):
                raw = stage.tile([128, 2, EP, cols], E4M3, tag=f"{name}{c0}")
                off = 2 * EP * c0
                nc.sync.dma_start(
                    out=raw[:],
                    in_=src_d[:, off : off + 2 * EP * cols].rearrange(
                        "p (j e t) -> p j e t", j=2, e=EP),
                )
                return raw

            def proj_qk(raw, wi, dst16, col0, cols):
                ps = psP.tile([128, 512], F32, tag="ps")
                for ep in range(EP):
                    nc.tensor.matmul(
                        ps[:D, :cols],
                        lhsT=w8[:, :, wi, ep, :],
                        rhs=raw[:, :, ep, :],
                        start=(ep == 0),
                        stop=(ep == EP - 1),
                        perf_mode=DRow,
                    )
                nc.vector.tensor_copy(dst16[:, col0 : col0 + cols], ps[:D, :cols])

            def proj_v(raw, lk0, nkb):
                for t in range(nkb):
                    ps = psP.tile([128, 512], F32, tag="ps")
                    for ep in range(EP):
                        nc.tensor.matmul(
                            ps[:, :D],
                            lhsT=raw[:, :, ep, t * KB : (t + 1) * KB],
                            rhs=w8[:, :, 1, ep, :],
                            start=(ep == 0),
                            stop=(ep == EP - 1),
                            perf_mode=DRow,
                        )
                    if lk0 + t < L16:
                        nc.vector.tensor_copy(v16[:, lk0 + t, :D], ps[:, :D])
                    nc.vector.tensor_copy(v8p[:, lk0 + t, :D], ps[:, :D])

            # --- attention ----------------------------------------------
            def qk_exp_group(i, l0, nl, po):
                fp16pv = (i < I16) or not PV8
                pss = psA.tile([128, G, QB], F32, tag="pss")
                for u in range(nl):
                    l = l0 + u
                    nc.tensor.matmul(
                        pss[:, u, :],
                        lhsT=kT16[:, l * KB : (l + 1) * KB],
                        rhs=qT16[:, QB * i : QB * (i + 1)],
                        start=True,
                        stop=(l != i),
                    )
                    if l == i:
                        nc.tensor.matmul(
                            pss[:, u, :],
                            lhsT=idm[:],
                            rhs=mask_sb[:],
                            start=False,
                            stop=True,
                            perf_mode=DRow,
                        )
                if fp16pv:
                    pe = pe16_pool.tile([128, G, QB], F16, tag="pe16")
                else:
                    pe = pe8_pool.tile([128, G + 1, QB], E4M3, tag="pe8")
                    if nl % 2 == 1:   # odd tail pairs with the slot-G zeros
                        nc.vector.memset(pe[:, G, :], 0.0)
                nc.scalar.activation(
                    pe[:, :nl, :],
                    pss[:, :nl, :],
                    mybir.ActivationFunctionType.Exp,
                    bias=bias_sb[:],
                    scale=EXP_SCALE,
                )

                def pv():
                    if fp16pv:
                        for half in (0, 1):
                            for u in range(nl):
                                l = l0 + u
                                nc.tensor.matmul(
                                    po[:, half, :],
                                    lhsT=pe[:, u, half * KB : (half + 1) * KB],
                                    rhs=v16[:, l, : D + 1],
                                    start=(l == 0 and half == 0),
                                    stop=(l == i and half == 1),
                                )
                    else:
                        for half in (0, 1):
                            u = 0
                            while u < nl:
                                if u + 1 < nl:
                                    pe_ap = pe[:, u : u + 2, half * KB : (half + 1) * KB]
                                    v_ap = v8p[:, l0 + u : l0 + u + 2, : D + 1]
                                else:   # odd tail: pair with zero slots
                                    pe_ap = pe[:, u : G + 1 : G - u, half * KB : (half + 1) * KB]
                                    v_ap = v8p[:, l0 + u : NLK + 1 : NLK - l0 - u, : D + 1]
                                nc.tensor.matmul(
                                    po[:, half, :],
                                    lhsT=pe_ap,
                                    rhs=v_ap,
                                    start=(l0 == 0 and u == 0 and half == 0),
                                    stop=(l0 + nl == i + 1 and u + 2 >= nl and half == 1),
                                    perf_mode=DRow,
                                )
                                u += 2
                    if l0 + nl == i + 1:
                        nc.vector.tensor_copy(od_sb[:, 2 * i : 2 * i + 2], po[:, :, D])
                        nc.vector.tensor_copy(on_sb[:, 2 * i : 2 * i + 2, :], po[:, :, :D])

                return pv

            # --- DMAs in need-order (serial DMA_ENGINES). Processing order
            # is blocks [2..15, 0, 1]: the fp16 q-hi/v-hi bytes for blocks
            # 0-1 ship late, out of the saturated early window.
            wb_r = wb_d.rearrange("p (w i d) -> p w i d", w=3, i=8)
            nc.sync.dma_start(out=wb[:, 0, :, :], in_=wb_r[:, 0, :, :])   # k w
            kh0 = load16(kh_d, "k", 0, K_HI[0])
            kh1 = load16(kh_d, "k", K_HI[0], K_HI[1])
            nc.sync.dma_start(out=w8[:], in_=w8_d.rearrange(
                "p (j w e d) -> p j w e d", j=2, w=3, e=EP))

            koff, qoff, voff = [0], [0], [0]
            kck, qck, vck = [], [], []

            def quec(lst, src_d, name, cols, acc):
                lst.append((load8(src_d, name, acc[0], cols), acc[0], cols))
                acc[0] += cols

            order = [
                ("k", 0), ("q", 0), ("q", 1),      # kb2-3, qb2, qb3
                ("q", 2), ("k", 1),                # qb4, kb4-7
                ("v", 0), ("v", 1),                # v8p kb0-1, v kb2-3
                ("q", 3), ("v", 2),                # qb5, v kb4-7
                ("q", 4), ("q", 5),                # qb6, qb7
                ("k", 2), ("q", 6),                # kb8-11, qb8
                ("v", 3), ("q", 7),                # v kb8-11, qb9
                ("q", 8), ("k", 3),                # qb10, kb12-15
                ("q", 9), ("v", 4),                # qb11, v kb12-15
                ("q", 10), ("q", 11),              # qb12, qb13
                ("q", 12), ("q", 13),              # qb14, qb15
            ]
            for kind, ci in order:
                if kind == "k":
                    quec(kck, kt_d, "k", K_CH[ci], koff)
                elif kind == "q":
                    quec(qck, qt_d, "q", Q_CH[ci], qoff)
                else:
                    quec(vck, vt_d, "v", V_CH[ci], voff)
            nc.sync.dma_start(out=wb[:, 1:3, :, :], in_=wb_r[:, 1:3, :, :])  # v,q w
            qh0 = load16(qh_d, "q", 0, Q_HI[0])
            qh1 = load16(qh_d, "q", Q_HI[0], Q_HI[1])
            vh0 = load16(vh_d, "v", 0, V_HI[0])

            # --- phase 0 projections (before the first block's QK)
            proj_qk_hi(kh0, 0, kT16, 0, K_HI[0])
            proj_qk_hi(kh1, 0, kT16, K_HI[0], K_HI[1])
            proj_qk(kck[0][0], 0, kT16, KHI + kck[0][1], kck[0][2])
            proj_qk(qck[0][0], 2, qT16, QHI + qck[0][1], qck[0][2])

            # --- projection jobs, keyed by PROCESSING POSITION (int = before
            # that position's QK; +.5 = after its last QK group)
            jobs = {}

            def at(key, fn):
                jobs.setdefault(key, []).append(fn)

            at(0.5, lambda: proj_v(vck[0][0], 0, 2))       # v8p kb0-1
            at(0.75, lambda: proj_qk(qck[1][0], 2, qT16, QHI + qck[1][1], qck[1][2]))
            at(1, lambda: proj_v(vck[1][0], 2, 2))         # v8p kb2-3
            at(1.5, lambda: proj_qk(kck[1][0], 0, kT16, KHI + kck[1][1], kck[1][2]))
            at(2, lambda: proj_qk(qck[2][0], 2, qT16, QHI + qck[2][1], qck[2][2]))
            at(2.5, lambda: proj_v(vck[2][0], 4, 4))       # v8p kb4-7
            at(3, lambda: proj_qk(qck[3][0], 2, qT16, QHI + qck[3][1], qck[3][2]))
            at(4, lambda: proj_qk(qck[4][0], 2, qT16, QHI + qck[4][1], qck[4][2]))
            at(5, lambda: proj_qk(qck[5][0], 2, qT16, QHI + qck[5][1], qck[5][2]))
            at(5.5, lambda: proj_qk(kck[2][0], 0, kT16, KHI + kck[2][1], kck[2][2]))
            at(6, lambda: proj_qk(qck[6][0], 2, qT16, QHI + qck[6][1], qck[6][2]))
            at(6.5, lambda: proj_v(vck[3][0], 8, 4))       # v8p kb8-11
            at(7, lambda: proj_qk(qck[7][0], 2, qT16, QHI + qck[7][1], qck[7][2]))
            at(8, lambda: proj_qk(qck[8][0], 2, qT16, QHI + qck[8][1], qck[8][2]))
            at(9, lambda: proj_qk(kck[3][0], 0, kT16, KHI + kck[3][1], kck[3][2]))
            at(9, lambda: proj_qk(qck[9][0], 2, qT16, QHI + qck[9][1], qck[9][2]))
            at(10, lambda: proj_v(vck[4][0], 12, 4))       # v8p kb12-15
            at(10, lambda: proj_qk(qck[10][0], 2, qT16, QHI + qck[10][1], qck[10][2]))
            at(11, lambda: proj_qk(qck[11][0], 2, qT16, QHI + qck[11][1], qck[11][2]))
            at(12, lambda: proj_qk(qck[12][0], 2, qT16, QHI + qck[12][1], qck[12][2]))
            at(12.5, lambda: proj_qk(qck[13][0], 2, qT16, QHI + qck[13][1], qck[13][2]))
            at(13, lambda: proj_qk_hi(qh0, 2, qT16, 0, Q_HI[0]))
            at(13.25, lambda: proj_qk_hi(qh1, 2, qT16, Q_HI[0], Q_HI[1]))
            at(13.5, lambda: proj_v_hi(vh0, 0, 2))

            BO = list(range(2, NQB)) + [1, 0]
            pvq = []
            for pos, i in enumerate(BO):
                for fn in jobs.get(pos, []):
                    fn()
                po = psO.tile([128, 2, D + 1], F32, tag="po")
                if i == NQB - 1:
                    glist = [(0, 4), (4, 4), (8, 4), (12, 3), (15, 1)]
                else:
                    glist = [(g * G, min(G, i + 1 - g * G))
                             for g in range((i + 1 + G - 1) // G)]
                for gi, (l0, nl) in enumerate(glist):
                    pvq.append(qk_exp_group(i, l0, nl, po))
                    if len(pvq) > 2:
                        pvq.pop(0)()
                    if gi == len(glist) - 1:
                        for fn in jobs.get(pos + 0.5, []):
                            fn()
                        for fn in jobs.get(pos + 0.75, []):
                            fn()
                    elif gi == 0:
                        for fn in jobs.get(pos + 0.25, []):
                            fn()
                while len(pvq) > 2:
                    pvq.pop(0)()
                if pos == 9:
                    nc.sync.dma_start(out=on_d[:, 4 * D : 20 * D], in_=on_sb[:, 4:20, :])
                elif pos == 13:
                    nc.sync.dma_start(out=on_d[:, 20 * D : 30 * D], in_=on_sb[:, 20:30, :])
                elif pos == 14:   # b15 + b1 tiles are done once b1's pv drains
                    nc.sync.dma_start(out=on_d[:, 30 * D : 32 * D], in_=on_sb[:, 30:32, :])
                    nc.sync.dma_start(out=od_d[:, 4:32], in_=od_sb[:, 4:32])
            while pvq:
                pvq.pop(0)()
            nc.sync.dma_start(out=on_d[:, 0 : 4 * D], in_=on_sb[:, 0:4, :])
            nc.sync.dma_start(out=od_d[:, 0:4], in_=od_sb[:, 0:4])

    nc.compile()
    return nc


def _host_shards(K, Q, V, Wk, Wq, Wv):
    E4np = ml_dtypes.float8_e4m3   # dt.float8e4 is IEEE e4m3 (max 240), NOT e4m3fn
    E5np = ml_dtypes.float8_e5m2

    def packw_folded(dt):
        out = np.empty((128, 2, 3, EP, D), dtype=np.float32)
        for wi, W in enumerate((Wk, Wv, Wq)):
            r = (WSCALE * W).reshape(EP, 2, 128, D)   # [ep, j, p, d]
            out[:, :, wi, :, :] = r.transpose(2, 1, 0, 3)
        return np.ascontiguousarray(out.reshape(128, -1)).astype(dt)

    def pack_hi(XT, chunks, dt):
        parts, c0 = [], 0
        for cols in chunks:
            blk = XT[:, c0 : c0 + cols].reshape(8, 128, cols)
            parts.append(blk.transpose(1, 0, 2).reshape(128, 8 * cols))
            c0 += cols
        return np.ascontiguousarray(np.concatenate(parts, axis=1)).astype(dt)

    def packw_classic(dt):
        mats = []
        for W in (Wk, Wv, Wq):
            mats.append(
                np.ascontiguousarray(
                    (WSCALE * W).reshape(8, 128, D).transpose(1, 0, 2).reshape(128, 8 * D)
                )
            )
        return np.concatenate(mats, axis=1).astype(dt)

    def pack_lo(XT, c_start, chunks, dt):
        parts, c0 = [], c_start
        for cols in chunks:
            blk = XT[:, c0 : c0 + cols].reshape(EP, 2, 128, cols)   # [ep, j, p, t]
            parts.append(blk.transpose(2, 1, 0, 3).reshape(128, 2 * EP * cols))
            c0 += cols
        return np.ascontiguousarray(np.concatenate(parts, axis=1)).astype(dt)

    w8 = packw_folded(E4np)
    wb = packw_classic(np.float16)

    in_maps = []
    for c in range(NCORES):
        b, h = c // 2, c % 2
        kidx = np.concatenate(
            [np.arange(KB * (2 * l + h), KB * (2 * l + h) + KB) for l in range(NLK)]
        )
        KT = np.ascontiguousarray(K[b][kidx].T)
        VT = np.ascontiguousarray(V[b][kidx].T)
        QT = np.ascontiguousarray(Q[b].T)
        r = np.arange(KB)[:, None] + h * KB
        cq = np.arange(QB)[None, :]
        mask = np.where(r > cq, np.float32(MASK_VAL), np.float32(0.0))
        mask2 = mask.reshape(2, 64, QB).transpose(1, 0, 2).reshape(64, 2 * QB)
        ident = 1024.0 * np.eye(128, dtype=np.float32)
        idm = ident.reshape(2, 64, 128).transpose(1, 0, 2).reshape(64, 2 * 128)
        in_maps.append(
            {
                "kh": pack_hi(KT[:, :KHI], K_HI, np.float16),
                "vh": pack_hi(VT[:, :VHI], V_HI, np.float16),
                "qh": pack_hi(QT[:, :QHI], Q_HI, np.float16),
                "qt": pack_lo(QT, QHI, Q_CH, E4np),
                "kt": pack_lo(KT, KHI, K_CH, E4np),
                "vt": pack_lo(VT, 0, V_CH, E4np),
                "wb": wb,
                "w8": w8,
                "idm": idm.astype(E5np),
                "mask": mask2.astype(E5np),
            }
        )
    return in_maps


def kernel(K, Q, V, Wk, Wq, Wv, _trace=False):
    K = np.asarray(K)
    Q = np.asarray(Q)
    V = np.asarray(V)
    Wk = np.asarray(Wk)
    Wq = np.asarray(Wq)
    Wv = np.asarray(Wv)

    if "nc" not in _CACHE:
        _CACHE["nc"] = _build_nc()
    nc = _CACHE["nc"]

    in_maps = _host_shards(K, Q, V, Wk, Wq, Wv)
    res = run_bass_kernel_spmd(
        nc, in_maps, core_ids=list(range(NCORES)), trace=_trace
    )
    _CACHE["last_result"] = res

    out = np.empty((B, T, D), dtype=np.float32)
    for b in range(B):
        ra, rb = res.results[2 * b], res.results[2 * b + 1]
        num = ra["on"].astype(np.float32) + rb["on"].astype(np.float32)
        den = ra["od"].astype(np.float32) + rb["od"].astype(np.float32)
        num = num.reshape(128, NQB * 2, D).transpose(1, 0, 2).reshape(T, D)
        den = den.reshape(128, NQB * 2).T.reshape(T, 1)
        out[b] = num / (WSCALE * den)
    return out


# revision 27
# speedup vs baseline: 1.0593x; 1.0593x over previous
"""Causal single-head attention on 8 TRN2 NeuronCores, v5 (DoubleRow + fp16 scores).

Problem: K,Q,V [4, 4096, 1024] f32, Wk/Wq/Wv [1024, 64] f32.
out[b,q,:] = softmax_causal((Q Wq)(K Wk)^T / 8) @ (V Wv)

Sharding: core c = 2b+h owns batch b = c//2 with key-parity h = c%2 (local
key block l = global block 2l+h). Each core emits numerator+denominator
[4096, 65]; the host sums the pair and divides.

Engine strategy: the exp stream on ScalarE (~36us busy) is the hard floor;
everything else is compressed far below it and scheduled around keeping
ScalarE fed:
 - x ships entirely as fp8e4 (IEEE e4m3, max 240) host-folded [128, 2, ...]
   so projections run DoubleRow with 256-row contraction (4x fewer PE
   cycles); weights are 32x-scaled so all on-chip values stay within e4m3
   range.
 - q/k are stored fp16 (scores wear quantization noise directly, so fp8
   storage would blow the 2e-2 error budget); QK is a plain fp16 matmul.
 - pe (exp output) is written e4m3 and PV pairs two key blocks per
   DoubleRow matmul (pe group slots are j-adjacent; v8 is padded to 80
   cols so slot strides stay %16==0); odd tails pair with zero slots.
 - the e5m2 DoubleRow mask matmul folds the causal diagonal into the score
   accumulation; exp is exp(score * 2^-13 - 2.5) on ScalarE.
All DRAM tensors are host-packed chunk-contiguous (one descriptor per
partition, full DMA rate); chunk sizes/order are tuned so each lands just
before its consumer block (DMA_ENGINES is a serial resource in the cost
model). The final block's output ships straight from PSUM to cut the tail.
"""

import ml_dtypes
import numpy as np

import concourse.mybir as mybir
import concourse.tile as tile
from concourse import bacc
from concourse.bass_utils import run_bass_kernel_spmd

B, T, E, D = 4, 4096, 1024, 64
NCORES = 8
QB = 256        # query block
KB = 128        # key block
NQB = T // QB   # 16 query blocks
NLK = T // KB // 2   # 16 local key blocks per core
G = 4           # key blocks per PSUM score group / exp call
EP = E // 256   # 4 folded e-pairs
I16 = 2         # blocks 0-1 use fp16 pe/v in PV (early-query insurance)
L16 = 2

F32 = mybir.dt.float32
F16 = mybir.dt.float16
E4M3 = mybir.dt.float8e4
E5M2 = mybir.dt.float8e5
DRow = mybir.MatmulPerfMode.DoubleRow

WSCALE = 32.0
EXP_SCALE = 0.125 / (WSCALE * WSCALE)   # 2^-13, exact
EXP_BIAS = -2.5
MASK_VAL = -14336.0   # e5m2-exact; * 1024 (ident) * 2^-13 = -1792 -> exp = 0

_CACHE = {}

PV8 = True    # e4m3 exp output + DoubleRow paired PV for blocks >= I16

# chunk column layouts (local cols). Early cols ship fp16 (early rows have
# large |out| and little softmax averaging, so they set the max-err); the
# bulk ships folded e4m3. Early chunks small so phase 0 starts fast.
K_HI = [128, 128]                     # kb0, kb1 (fp16)
Q_HI = [256, 256]                     # blocks 0, 1 (fp16)
V_HI = [256]                          # kb0-1 (fp16)
K_CH = [256, 512, 512, 512]           # kb2-3, 4-7, 8-11, 12-15 (e4m3)
Q_CH = [256] * 14                     # blocks 2..15, one chunk each
V_CH = [256, 256, 512, 512, 512]      # kb0-1(v8p), 2-3, 4-7, 8-11, 12-15
KHI = sum(K_HI)
QHI = sum(Q_HI)
VHI = sum(V_HI)


def _build_nc():
    nc = bacc.Bacc()
    kh_d = nc.declare_dram_parameter("kh", [128, 8 * KHI], F16, isOutput=False)
    vh_d = nc.declare_dram_parameter("vh", [128, 8 * VHI], F16, isOutput=False)
    qh_d = nc.declare_dram_parameter("qh", [128, 8 * QHI], F16, isOutput=False)
    qt_d = nc.declare_dram_parameter("qt", [128, 2 * EP * (T - QHI)], E4M3, isOutput=False)
    kt_d = nc.declare_dram_parameter("kt", [128, 2 * EP * (T // 2 - KHI)], E4M3, isOutput=False)
    vt_d = nc.declare_dram_parameter("vt", [128, 2 * EP * (T // 2)], E4M3, isOutput=False)
    wb_d = nc.declare_dram_parameter("wb", [128, 3 * 8 * D], F16, isOutput=False)
    w8_d = nc.declare_dram_parameter("w8", [128, 2 * 3 * EP * D], E4M3, isOutput=False)
    idm_d = nc.declare_dram_parameter("idm", [64, 2 * 128], E5M2, isOutput=False)
    mask_d = nc.declare_dram_parameter("mask", [64, 2 * QB], E5M2, isOutput=False)
    on_d = nc.declare_dram_parameter("on", [128, NQB * 2 * D], mybir.dt.bfloat16, isOutput=True)
    od_d = nc.declare_dram_parameter("od", [128, NQB * 2], F32, isOutput=True)

    with tile.TileContext(nc) as tc:
        with (
            tc.tile_pool(name="w", bufs=1) as wpool,
            tc.tile_pool(name="res", bufs=1) as res,
            tc.tile_pool(name="stage", bufs=1) as stage,
            tc.tile_pool(name="pe16", bufs=2) as pe16_pool,
            tc.tile_pool(name="pe8", bufs=7) as pe8_pool,
            tc.tile_pool(name="psP", bufs=2, space="PSUM") as psP,
            tc.tile_pool(name="psA", bufs=2, space="PSUM") as psA,
            tc.tile_pool(name="psO", bufs=2, space="PSUM") as psO,
        ):
            w8 = wpool.tile([128, 2, 3, EP, D], E4M3, tag="w8")
            wb = wpool.tile([128, 3, 8, D], F16, tag="wb")
            idm = wpool.tile([64, 2, 128], E5M2, tag="idm")
            mask_sb = wpool.tile([64, 2, QB], E5M2, tag="mask")
            bias_sb = wpool.tile([128, 1], F32, tag="bias")
            warm_sb = wpool.tile([64, 16], F16, tag="warm")
            nc.vector.memset(bias_sb[:], EXP_BIAS)
            nc.vector.memset(warm_sb[:], 0.0)

            qT16 = res.tile([64, T], F16, tag="qT16")
            kT16 = res.tile([64, T // 2], F16, tag="kT16")
            v16 = res.tile([128, L16, D + 1], F16, tag="v16")
            v8p = res.tile([128, NLK + 1, 80], E4M3, tag="v8p")
            on_sb = res.tile([128, NQB * 2, D], mybir.dt.bfloat16, tag="on")
            od_sb = res.tile([128, NQB * 2], F32, tag="od")

            # PE warm-up: tiny matmuls at t~0 so the p-state ramp (3us from
            # first PE activity) finishes before the first real matmul.
            ps_warm = psP.tile([128, 512], F32, tag="ps")
            for r in range(6):
                nc.tensor.matmul(
                    ps_warm[:16, :16], lhsT=warm_sb[:, :16], rhs=warm_sb[:, :16],
                    start=True, stop=True,
                )

            nc.vector.memset(v16[:, :, D : D + 1], 1.0)
            nc.vector.memset(v8p[:, 0:NLK, D : D + 1], 1.0)
            nc.vector.memset(v8p[:, NLK, :], 0.0)

            nc.gpsimd.dma_start(out=idm[:], in_=idm_d.rearrange("p (two m) -> p two m", two=2))
            nc.gpsimd.dma_start(out=mask_sb[:], in_=mask_d.rearrange("p (two m) -> p two m", two=2))

            def load16(src_d, name, c0, cols):
                raw = stage.tile([128, 8, cols], F16, tag=f"{name}h{c0}")
                off = 8 * c0
                nc.sync.dma_start(
                    out=raw[:],
                    in_=src_d[:, off : off + 8 * cols].rearrange(
                        "p (i t) -> p i t", i=8),
                )
                return raw

            def proj_qk_hi(raw, wi, dst16, col0, cols):
                ps = psP.tile([128, 512], F32, tag="ps")
                for i in range(8):
                    nc.tensor.matmul(
                        ps[:D, :cols],
                        lhsT=wb[:, wi, i, :],
                        rhs=raw[:, i, :],
                        start=(i == 0),
                        stop=(i == 7),
                    )
                nc.vector.tensor_copy(dst16[:, col0 : col0 + cols], ps[:D, :cols])

            def proj_v_hi(raw, lk0, nkb):
                for t in range(nkb):
                    ps = psP.tile([128, 512], F32, tag="ps")
                    for i in range(8):
                        nc.tensor.matmul(
                            ps[:, :D],
                            lhsT=raw[:, i, t * KB : (t + 1) * KB],
                            rhs=wb[:, 1, i, :],
                            start=(i == 0),
                            stop=(i == 7),
                        )
                    if lk0 + t < L16:
                        nc.vector.tensor_copy(v16[:, lk0 + t, :D], ps[:, :D])
                    nc.vector.tensor_copy(v8p[:, lk0 + t, :D], ps[:, :D])

            def load8(src_d, name, c0, cols):
                raw = stage.tile([128, 2, EP, cols], E4M3, tag=f"{name}{c0}")
                off = 2 * EP * c0
                nc.sync.dma_start(
                    out=raw[:],
                    in_=src_d[:, off : off + 2 * EP * cols].rearrange(
                        "p (j e t) -> p j e t", j=2, e=EP),
                )
                return raw

            def proj_qk(raw, wi, dst16, col0, cols):
                ps = psP.tile([128, 512], F32, tag="ps")
                for ep in range(EP):
                    nc.tensor.matmul(
                        ps[:D, :cols],
                        lhsT=w8[:, :, wi, ep, :],
                        rhs=raw[:, :, ep, :],
                        start=(ep == 0),
                        stop=(ep == EP - 1),
                        perf_mode=DRow,
                    )
                nc.vector.tensor_copy(dst16[:, col0 : col0 + cols], ps[:D, :cols])

            def proj_v(raw, lk0, nkb):
                for t in range(nkb):
                    ps = psP.tile([128, 512], F32, tag="ps")
                    for ep in range(EP):
                        nc.tensor.matmul(
                            ps[:, :D],
                            lhsT=raw[:, :, ep, t * KB : (t + 1) * KB],
                            rhs=w8[:, :, 1, ep, :],
                            start=(ep == 0),
                            stop=(ep == EP - 1),
                            perf_mode=DRow,
                        )
                    if lk0 + t < L16:
                        nc.vector.tensor_copy(v16[:, lk0 + t, :D], ps[:, :D])
                    nc.vector.tensor_copy(v8p[:, lk0 + t, :D], ps[:, :D])

            # --- attention ----------------------------------------------
            def qk_exp_group(i, l0, nl, po):
                fp16pv = (i < I16) or not PV8
                pss = psA.tile([128, G, QB], F32, tag="pss")
                for u in range(nl):
                    l = l0 + u
                    nc.tensor.matmul(
                        pss[:, u, :],
                        lhsT=kT16[:, l * KB : (l + 1) * KB],
                        rhs=qT16[:, QB * i : QB * (i + 1)],
                        start=True,
                        stop=(l != i),
                    )
                    if l == i:
                        nc.tensor.matmul(
                            pss[:, u, :],
                            lhsT=idm[:],
                            rhs=mask_sb[:],
                            start=False,
                            stop=True,
                            perf_mode=DRow,
                        )
                if fp16pv:
                    pe = pe16_pool.tile([128, G, QB], F16, tag="pe16")
                else:
                    pe = pe8_pool.tile([128, G + 1, QB], E4M3, tag="pe8")
                    if nl % 2 == 1:   # odd tail pairs with the slot-G zeros
                        nc.vector.memset(pe[:, G, :], 0.0)
                nc.scalar.activation(
                    pe[:, :nl, :],
                    pss[:, :nl, :],
                    mybir.ActivationFunctionType.Exp,
                    bias=bias_sb[:],
                    scale=EXP_SCALE,
                )

                def pv():
                    if fp16pv:
                        for half in (0, 1):
                            for u in range(nl):
                                l = l0 + u
                                nc.tensor.matmul(
                                    po[:, half, :],
                                    lhsT=pe[:, u, half * KB : (half + 1) * KB],
                                    rhs=v16[:, l, : D + 1],
                                    start=(l == 0 and half == 0),
                                    stop=(l == i and half == 1),
                                )
                    else:
                        for half in (0, 1):
                            u = 0
                            while u < nl:
                                if u + 1 < nl:
                                    pe_ap = pe[:, u : u + 2, half * KB : (half + 1) * KB]
                                    v_ap = v8p[:, l0 + u : l0 + u + 2, : D + 1]
                                else:   # odd tail: pair with zero slots
                                    pe_ap = pe[:, u : G + 1 : G - u, half * KB : (half + 1) * KB]
                                    v_ap = v8p[:, l0 + u : NLK + 1 : NLK - l0 - u, : D + 1]
                                nc.tensor.matmul(
                                    po[:, half, :],
                                    lhsT=pe_ap,
                                    rhs=v_ap,
                                    start=(l0 == 0 and u == 0 and half == 0),
                                    stop=(l0 + nl == i + 1 and u + 2 >= nl and half == 1),
                                    perf_mode=DRow,
                                )
                                u += 2
                    if l0 + nl == i + 1:
                        nc.vector.tensor_copy(od_sb[:, 2 * i : 2 * i + 2], po[:, :, D])
                        nc.vector.tensor_copy(on_sb[:, 2 * i : 2 * i + 2, :], po[:, :, :D])

                return pv

            # --- DMAs in need-order (serial DMA_ENGINES). Processing order
            # is blocks [2..15, 0, 1]: the fp16 q-hi/v-hi bytes for blocks
            # 0-1 ship late, out of the saturated early window.
            wb_r = wb_d.rearrange("p (w i d) -> p w i d", w=3, i=8)
            nc.sync.dma_start(out=wb[:, 0, :, :], in_=wb_r[:, 0, :, :])   # k w
            kh0 = load16(kh_d, "k", 0, K_HI[0])
            kh1 = load16(kh_d, "k", K_HI[0], K_HI[1])
            nc.sync.dma_start(out=w8[:], in_=w8_d.rearrange(
                "p (j w e d) -> p j w e d", j=2, w=3, e=EP))

            koff, qoff, voff = [0], [0], [0]
            kck, qck, vck = [], [], []

            def quec(lst, src_d, name, cols, acc):
                lst.append((load8(src_d, name, acc[0], cols), acc[0], cols))
                acc[0] += cols

            order = [
                ("k", 0), ("q", 0), ("q", 1),      # kb2-3, qb2, qb3
                ("q", 2), ("k", 1),                # qb4, kb4-7
                ("v", 0), ("v", 1),                # v8p kb0-1, v kb2-3
                ("q", 3), ("v", 2),                # qb5, v kb4-7
                ("q", 4), ("q", 5),                # qb6, qb7
                ("k", 2), ("q", 6),                # kb8-11, qb8
                ("v", 3), ("q", 7),                # v kb8-11, qb9
                ("q", 8), ("k", 3),                # qb10, kb12-15
                ("q", 9), ("v", 4),                # qb11, v kb12-15
                ("q", 10), ("q", 11),              # qb12, qb13
                ("q", 12), ("q", 13),              # qb14, qb15
            ]
            for kind, ci in order:
                if kind == "k":
                    quec(kck, kt_d, "k", K_CH[ci], koff)
                elif kind == "q":
                    quec(qck, qt_d, "q", Q_CH[ci], qoff)
                else:
                    quec(vck, vt_d, "v", V_CH[ci], voff)
            nc.sync.dma_start(out=wb[:, 1:3, :, :], in_=wb_r[:, 1:3, :, :])  # v,q w
            qh0 = load16(qh_d, "q", 0, Q_HI[0])
            qh1 = load16(qh_d, "q", Q_HI[0], Q_HI[1])
            vh0 = load16(vh_d, "v", 0, V_HI[0])

            # --- phase 0 projections (before the first block's QK)
            proj_qk_hi(kh0, 0, kT16, 0, K_HI[0])
            proj_qk_hi(kh1, 0, kT16, K_HI[0], K_HI[1])
            proj_qk(kck[0][0], 0, kT16, KHI + kck[0][1], kck[0][2])
            proj_qk(qck[0][0], 2, qT16, QHI + qck[0][1], qck[0][2])

            # --- projection jobs, keyed by PROCESSING POSITION (int = before
            # that position's QK; +.5 = after its last QK group)
            jobs = {}

            def at(key, fn):
                jobs.setdefault(key, []).append(fn)

            at(0.5, lambda: proj_qk(qck[1][0], 2, qT16, QHI + qck[1][1], qck[1][2]))
            at(1, lambda: proj_qk(qck[2][0], 2, qT16, QHI + qck[2][1], qck[2][2]))
            at(1.5, lambda: proj_v(vck[0][0], 0, 2))
            at(1.5, lambda: proj_qk(kck[1][0], 0, kT16, KHI + kck[1][1], kck[1][2]))
            at(2.25, lambda: proj_v(vck[1][0], 2, 2))
            at(2.5, lambda: proj_qk(qck[3][0], 2, qT16, QHI + qck[3][1], qck[3][2]))
            at(3, lambda: proj_v(vck[2][0], 4, 4))
            at(3.5, lambda: proj_qk(qck[4][0], 2, qT16, QHI + qck[4][1], qck[4][2]))
            at(4.5, lambda: proj_qk(qck[5][0], 2, qT16, QHI + qck[5][1], qck[5][2]))
            at(5.5, lambda: proj_qk(kck[2][0], 0, kT16, KHI + kck[2][1], kck[2][2]))
            at(6, lambda: proj_qk(qck[6][0], 2, qT16, QHI + qck[6][1], qck[6][2]))
            at(6.5, lambda: proj_v(vck[3][0], 8, 4))
            at(7, lambda: proj_qk(qck[7][0], 2, qT16, QHI + qck[7][1], qck[7][2]))
            at(8, lambda: proj_qk(qck[8][0], 2, qT16, QHI + qck[8][1], qck[8][2]))
            at(9, lambda: proj_qk(kck[3][0], 0, kT16, KHI + kck[3][1], kck[3][2]))
            at(9, lambda: proj_qk(qck[9][0], 2, qT16, QHI + qck[9][1], qck[9][2]))
            at(10, lambda: proj_v(vck[4][0], 12, 4))
            at(10, lambda: proj_qk(qck[10][0], 2, qT16, QHI + qck[10][1], qck[10][2]))
            at(11, lambda: proj_qk(qck[11][0], 2, qT16, QHI + qck[11][1], qck[11][2]))
            at(12, lambda: proj_qk(qck[12][0], 2, qT16, QHI + qck[12][1], qck[12][2]))
            at(12.5, lambda: proj_qk(qck[13][0], 2, qT16, QHI + qck[13][1], qck[13][2]))
            at(13, lambda: proj_qk_hi(qh0, 2, qT16, 0, Q_HI[0]))
            at(13.25, lambda: proj_qk_hi(qh1, 2, qT16, Q_HI[0], Q_HI[1]))
            at(13.5, lambda: proj_v_hi(vh0, 0, 2))

            BO = list(range(2, NQB)) + [1, 0]
            pvq = []
            for pos, i in enumerate(BO):
                for fn in jobs.get(pos, []):
                    fn()
                po = psO.tile([128, 2, D + 1], F32, tag="po")
                if i == NQB - 1:
                    glist = [(0, 4), (4, 4), (8, 4), (12, 3), (15, 1)]
                else:
                    glist = [(g * G, min(G, i + 1 - g * G))
                             for g in range((i + 1 + G - 1) // G)]
                for gi, (l0, nl) in enumerate(glist):
                    pvq.append(qk_exp_group(i, l0, nl, po))
                    if len(pvq) > 2:
                        pvq.pop(0)()
                    if gi == len(glist) - 1:
                        for fn in jobs.get(pos + 0.5, []):
                            fn()
                        for fn in jobs.get(pos + 0.75, []):
                            fn()
                    elif gi == 0:
                        for fn in jobs.get(pos + 0.25, []):
                            fn()
                while len(pvq) > 2:
                    pvq.pop(0)()
                if pos == 9:
                    nc.sync.dma_start(out=on_d[:, 4 * D : 20 * D], in_=on_sb[:, 4:20, :])
                elif pos == 13:
                    nc.sync.dma_start(out=on_d[:, 20 * D : 30 * D], in_=on_sb[:, 20:30, :])
                elif pos == 14:   # b15 + b1 tiles are done once b1's pv drains
                    nc.sync.dma_start(out=on_d[:, 30 * D : 32 * D], in_=on_sb[:, 30:32, :])
                    nc.sync.dma_start(out=od_d[:, 4:32], in_=od_sb[:, 4:32])
            while pvq:
                pvq.pop(0)()
            nc.sync.dma_start(out=on_d[:, 0 : 4 * D], in_=on_sb[:, 0:4, :])
            nc.sync.dma_start(out=od_d[:, 0:4], in_=od_sb[:, 0:4])

    nc.compile()
    return nc


def _host_shards(K, Q, V, Wk, Wq, Wv):
    E4np = ml_dtypes.float8_e4m3   # dt.float8e4 is IEEE e4m3 (max 240), NOT e4m3fn
    E5np = ml_dtypes.float8_e5m2

    def packw_folded(dt):
        out = np.empty((128, 2, 3, EP, D), dtype=np.float32)
        for wi, W in enumerate((Wk, Wv, Wq)):
            r = (WSCALE * W).reshape(EP, 2, 128, D)   # [ep, j, p, d]
            out[:, :, wi, :, :] = r.transpose(2, 1, 0, 3)
        return np.ascontiguousarray(out.reshape(128, -1)).astype(dt)

    def pack_hi(XT, chunks, dt):
        parts, c0 = [], 0
        for cols in chunks:
            blk = XT[:, c0 : c0 + cols].reshape(8, 128, cols)
            parts.append(blk.transpose(1, 0, 2).reshape(128, 8 * cols))
            c0 += cols
        return np.ascontiguousarray(np.concatenate(parts, axis=1)).astype(dt)

    def packw_classic(dt):
        mats = []
        for W in (Wk, Wv, Wq):
            mats.append(
                np.ascontiguousarray(
                    (WSCALE * W).reshape(8, 128, D).transpose(1, 0, 2).reshape(128, 8 * D)
                )
            )
        return np.concatenate(mats, axis=1).astype(dt)

    def pack_lo(XT, c_start, chunks, dt):
        parts, c0 = [], c_start
        for cols in chunks:
            blk = XT[:, c0 : c0 + cols].reshape(EP, 2, 128, cols)   # [ep, j, p, t]
            parts.append(blk.transpose(2, 1, 0, 3).reshape(128, 2 * EP * cols))
            c0 += cols
        return np.ascontiguousarray(np.concatenate(parts, axis=1)).astype(dt)

    w8 = packw_folded(E4np)
    wb = packw_classic(np.float16)

    in_maps = []
    for c in range(NCORES):
        b, h = c // 2, c % 2
        kidx = np.concatenate(
            [np.arange(KB * (2 * l + h), KB * (2 * l + h) + KB) for l in range(NLK)]
        )
        KT = np.ascontiguousarray(K[b][kidx].T)
        VT = np.ascontiguousarray(V[b][kidx].T)
        QT = np.ascontiguousarray(Q[b].T)
        r = np.arange(KB)[:, None] + h * KB
        cq = np.arange(QB)[None, :]
        mask = np.where(r > cq, np.float32(MASK_VAL), np.float32(0.0))
        mask2 = mask.reshape(2, 64, QB).transpose(1, 0, 2).reshape(64, 2 * QB)
        ident = 1024.0 * np.eye(128, dtype=np.float32)
        idm = ident.reshape(2, 64, 128).transpose(1, 0, 2).reshape(64, 2 * 128)
        in_maps.append(
            {
                "kh": pack_hi(KT[:, :KHI], K_HI, np.float16),
                "vh": pack_hi(VT[:, :VHI], V_HI, np.float16),
                "qh": pack_hi(QT[:, :QHI], Q_HI, np.float16),
                "qt": pack_lo(QT, QHI, Q_CH, E4np),
                "kt": pack_lo(KT, KHI, K_CH, E4np),
                "vt": pack_lo(VT, 0, V_CH, E4np),
                "wb": wb,
                "w8": w8,
                "idm": idm.astype(E5np),
                "mask": mask2.astype(E5np),
            }
        )
    return in_maps


def kernel(K, Q, V, Wk, Wq, Wv, _trace=False):
    K = np.asarray(K)
    Q = np.asarray(Q)
    V = np.asarray(V)
    Wk = np.asarray(Wk)
    Wq = np.asarray(Wq)
    Wv = np.asarray(Wv)

    if "nc" not in _CACHE:
        _CACHE["nc"] = _build_nc()
    nc = _CACHE["nc"]

    in_maps = _host_shards(K, Q, V, Wk, Wq, Wv)
    res = run_bass_kernel_spmd(
        nc, in_maps, core_ids=list(range(NCORES)), trace=_trace
    )
    _CACHE["last_result"] = res

    out = np.empty((B, T, D), dtype=np.float32)
    for b in range(B):
        ra, rb = res.results[2 * b], res.results[2 * b + 1]
        num = ra["on"].astype(np.float32) + rb["on"].astype(np.float32)
        den = ra["od"].astype(np.float32) + rb["od"].astype(np.float32)
        num = num.reshape(128, NQB * 2, D).transpose(1, 0, 2).reshape(T, D)
        den = den.reshape(128, NQB * 2).T.reshape(T, 1)
        out[b] = num / (WSCALE * den)
    return out
